# revision 31
# baseline (speedup 1.0000x reference)
"""Causal multi-head attention (nn_Attention_87840671138123) on 8 trn2 NeuronCores.

Problem (B=2, S=2048, D=1024, H=16 heads, E=64 head_dim), fp32:
    Q = einsum('bsd,hde->bhse', q, W_q)   (same for K, V)
    scores = Q @ K^T / sqrt(D), causal mask, softmax
    attn = probs @ V  -> [B, S, D] (head-major concat)
    out = attn @ W_o.T

Sharding: core = 4*b + quad. Each core handles batch b and a quad of 4 heads
(heads 4*quad .. 4*quad+3). It computes a partial output
    out_part = attn_quad @ W_o.T[quad rows, :]   [S, D]
and the host sums the 4 partials per batch (the "all-reduce" of the output
projection done host-side at gather time).

Device layout choices (per core):
 - Host passes xT = x[b].T  [D, S] so the d-contraction sits on partitions.
 - Projections produce QT/KT in "transposed" layout [head-pair x 64, S]
   (head h2 of a pair occupies partitions 64*h2..64*h2+63), and V in natural
   [t, e] layout augmented with a ones-column (V_aug [t, 65]) so the
   attn matmul also accumulates the softmax denominator as row 64.
 - scoresT[t, s] = (KT chunk).T @ QT  -> exp on ACT (scale 1/32 folded in)
   -> causal handled by (a) skipping fully-masked blocks, (b) shrinking the
   moving dim to the valid s-range for diagonal blocks, (c) one [128,128]
   triangular mask multiply for the diagonal 128-col strip.
 - attnT_aug[65, s] += V_aug.T @ expT accumulated over t chunks in PSUM.
 - Normalize: DVE copy of the denominator rows out of PSUM -> one DMA hop
   to partition 0 -> reciprocal_approx_fast (single custom-DVE op, ~5x
   faster than exact reciprocal; HW requires SBUF input at base partition
   0) -> gpsimd partition_broadcast -> one multiply per head into attnG.
 - Output projection: out[s, :] = sum_g (attnT chunk).T @ W_o.T slice,
   emitted one s-tile behind; partials stored fp16, host sums in fp32.

Numerics: Q/K path bf16 (score errors are absolute-small and only perturb
softmax weights, <~3e-4 on the output); V path fp16. Total ~6e-4 vs the
2e-2 gate.

Measured dead ends (all correct but slower on HW, rel to ~190us mean):
 - fp8 e4m3 + DoubleRow Q/K projections: 2x MAC rate trips the HAM power
   throttle (k=4 half-clock for ~107us vs ~31us baseline) -> +45us.
 - Merging the h2-pair exps into one 2-bank ACTIVATE: -20us of ACT time
   but coarsens the PE/ACT pipeline beat (PE idles ~1.2us per block,
   which also re-triggers the throttle) -> net +10us.
 - K=64 half-array score matmuls (no zero-padding): HAM reads them as
   half-idle and drops the clock -> +14us.
 - walrus --enable-ldw-opt=true: rejects this kernel's Ldweights even
   after a post-compile pass hoisted every semaphore wait onto an
   EventSemaphore before the load (all 640 LDWs verified wait-free in
   the BIR); the incompatibility is deeper in codegen -> no-compile.
   The ~30us of weight-load serialization is locked out.
 - Depth-2 input prefetch (tiles 0+1 upfront, j+2 in-loop), isolated:
   neutral (189.7/184.2 vs the checkpoint band 187.6-193.8/183.8-185.1
   max/mean) -- the traced 7.4us j=1 input wait trades evenly against
   DMA contention. The +27us regression originally blamed on this was
   actually the scalar-queue DMA triggers + ot bufs change (see below).
   Depth 1 kept.
 - gpsimd tensor_copy for the partition hop (it CAN cross partition
   bases, unlike DVE): ~6us per [1,1024] row -> +50us vs the DMA hop.
"""

import ml_dtypes
import numpy as np

import concourse.bass as bass
import concourse.tile as tile
from concourse import bacc, mybir
from concourse.bass_utils import run_bass_kernel_spmd

B, S, D, H, E = 2, 2048, 1024, 16, 64
P = 128
NCORES = 8
SJ = 512            # s-tile width
NJ = S // SJ        # 4 s-tiles
ND = D // P         # 8 d-chunks
NT = S // P         # 16 t-chunks
f32 = mybir.dt.float32
f32r = mybir.dt.float32r
bf16 = mybir.dt.bfloat16
fp16 = mybir.dt.float16
EXP = mybir.ActivationFunctionType.Exp
MULT = mybir.AluOpType.mult

fp8 = mybir.dt.float8e4
# fp8+DoubleRow projections were tried and are numerically fine (3e-3) but
# the 2x MAC rate trips the HAM power throttle (k=4 for ~107us vs ~31us),
# a large net loss. Keep the Q/K path in bf16.
QP_DT = bf16        # dtype of q/k inputs and Wq/Wk (projection matmuls)
QK_DT = bf16        # dtype of QT/KT (scores matmul)
V_DT = fp16         # dtype of v input, Wv, V_aug, expT, attnG, WoT
WSCALE = 1.0

_NP_OF = {bf16: ml_dtypes.bfloat16, fp16: np.float16, f32r: np.float32,
          f32: np.float32, fp8: ml_dtypes.float8_e4m3}

_NC_CACHE = []


def _patch_ldw_opt():
    """Enable walrus LDWEIGHTS optimization (fast weight load).

    bass_utils.bir_verify_and_optimise hardcodes --enable-ldw-opt=false;
    wrap it to rewrite the flag. Verified numerically by the test harness.
    """
    from concourse import bass_utils as _bu
    if getattr(_bu, "_ldw_patched", False):
        return
    _orig_run = _bu.run_command

    def _run(argv, **kw):
        argv = ["--enable-ldw-opt=true" if a == "--enable-ldw-opt=false" else a
                for a in argv]
        return _orig_run(argv, **kw)

    _bu.run_command = _run
    _bu._ldw_patched = True


def _build():
    nc = bacc.Bacc("TRN2", target_bir_lowering=False, debug=False)

    qT_d = nc.dram_tensor("qT", [D, S], QP_DT, kind="ExternalInput")
    kT_d = nc.dram_tensor("kT", [D, S], QP_DT, kind="ExternalInput")
    vT_d = nc.dram_tensor("vT", [D, S], V_DT, kind="ExternalInput")
    wq_d = nc.dram_tensor("wq", [D, 4 * E], QP_DT, kind="ExternalInput")
    wk_d = nc.dram_tensor("wk", [D, 4 * E], QP_DT, kind="ExternalInput")
    wv_d = nc.dram_tensor("wv", [D, 4 * E], V_DT, kind="ExternalInput")
    wot_d = nc.dram_tensor("wot", [4 * E, D], V_DT, kind="ExternalInput")
    tri_d = nc.dram_tensor("tri", [P, P], V_DT, kind="ExternalInput")
    # fp16 partials: host sums 4 per batch in fp32; quantization ~5e-4 rel,
    # well inside the 2e-2 gate, and halves both the DVE copy and out DMA.
    out_d = nc.dram_tensor("out", [S, D], fp16, kind="ExternalOutput")
    # j=3's output projection is split into two host-summed partials so the
    # g=0 half can run during attn(1,3) instead of serializing behind the
    # last epilogue: out rows 3*SJ.. get only the g=1 contribution, out2
    # carries the g=0 contribution.
    out2_d = nc.dram_tensor("out2", [SJ, D], fp16, kind="ExternalOutput")

    with tile.TileContext(nc) as tc:
        with (
            tc.tile_pool(name="pers", bufs=1) as pers,
            tc.tile_pool(name="ex", bufs=3) as ex_pool,
            tc.tile_pool(name="sm", bufs=2) as sm_pool,
            tc.tile_pool(name="ot", bufs=2) as ot_pool,
            tc.tile_pool(name="pj", bufs=2, space="PSUM") as pj_pool,
            tc.tile_pool(name="sc", bufs=2, space="PSUM") as sc_pool,
            tc.tile_pool(name="at", bufs=4, space="PSUM") as at_pool,
        ):
            # ---- persistent weights / constants ----
            # weights ride the vector engine's DMA queue, chunked, so the
            # first projection matmul waits on ~64KB, in parallel with the
            # j=0 input chunks on the sync queue
            wq_sb = [pers.tile([P, 2, 4 * E], QP_DT, name=f"wq_sb{i}")
                     for i in range(ND // 2)]
            wk_sb = [pers.tile([P, 2, 4 * E], QP_DT, name=f"wk_sb{i}")
                     for i in range(ND // 2)]
            wv_sb = pers.tile([P, ND, 4 * E], V_DT, name="wv_sb")
            for c in range(0, ND, 2):
                nc.scalar.dma_start(
                    wq_sb[c // 2][:],
                    wq_d.ap().rearrange("(o p) m -> p o m", p=P)[:, c:c + 2, :])
            for c in range(0, ND, 2):
                nc.scalar.dma_start(
                    wk_sb[c // 2][:],
                    wk_d.ap().rearrange("(o p) m -> p o m", p=P)[:, c:c + 2, :])
            nc.scalar.dma_start(
                wv_sb[:], wv_d.ap().rearrange("(o p) m -> p o m", p=P))
            wot_sb = pers.tile([P, 2, D], V_DT, name="wot_sb")
            nc.scalar.dma_start(
                wot_sb[:], wot_d.ap().rearrange("(g p) n -> p g n", p=P))
            tri_sb = pers.tile([P, P], V_DT, name="tri_sb")
            nc.scalar.dma_start(tri_sb[:], tri_d.ap())
            # all-ones column block for the PE-matmul partition broadcast in
            # the last epilogue (row 64 holds the stationary [1, E] slice so
            # its base partition matches the denominator staging row)
            ones_sb = pers.tile([P, E], bf16, name="ones_sb")
            nc.vector.memset(ones_sb[:], 1.0)

            # ---- persistent activations ----
            QT = [pers.tile([P, S], QK_DT, name=f"QT{g}") for g in range(2)]
            # Per-head KT zero-padded to 128 partitions: rows 0..63 hold the
            # head's K^T, rows 64..127 are zeros. The scores matmul then runs
            # with K=128 (full PE rows) -- the zero rows null out the other
            # head's Q rows in the shared QT rhs. Full-array matmuls keep the
            # HAM activity monitor from throttling the PE clock to 1.2 GHz
            # (K=64 / M=65 matmuls read as "half idle").
            KTH = [[pers.tile([P, S], QK_DT, name=f"KT{g}{h2}") for h2 in range(2)]
                   for g in range(2)]
            # V_aug is 65 cols [64 V | ones]: each head's attn PSUM holds
            # values on rows 0:64 and the softmax denominator on row 64.
            V = [pers.tile([P, NT, 2, E + 1], V_DT, name=f"V{g}") for g in range(2)]
            attnG = [pers.tile([P, S], V_DT, name=f"attnG{g}") for g in range(2)]
            for g in range(2):
                # pad rows are true zeros: every throttle observation this
                # session says the HAM reacts to switching power (fp8 2x MAC
                # rate tripled the k=4 time), so dead rows should be quiet.
                # (K=64 half-array score matmuls without padding were tried:
                # ~14us slower -- the clock drops on half-idle tiles.)
                nc.vector.memset(KTH[g][0][E:2 * E, :], 0.0)
                nc.vector.memset(KTH[g][1][0:E, :], 0.0)
                nc.vector.memset(V[g][:, :, :, E:E + 1], 1.0)

            # ---- software-pipelined schedule ----
            # Projections run one s-tile AHEAD of attention and are split so
            # PE always has independent work to chew on while each (g, j)
            # normalize epilogue releases the single in-flight atp pair
            # (at_pool bufs=2):
            #   attn(g0,j) epi(g0,j) [projQK(j+1)] attn(g1,j) epi(g1,j)
            #   [projV(j+1) + Wo(j-1)] -> next j

            def emit_loads(j):
                # all inputs are prefetched up front into persistent tiles
                # (they fit: 12MB of inputs + ~8MB of weights/activations vs
                # 26MB SBUF), so the j>=1 projections never wait on DMA and
                # the sync queue drains before the output DMAs start
                js = slice(j * SJ, (j + 1) * SJ)
                if j <= 1:
                    # separate per-pair tiles: dependency tracking is
                    # per-tile, so this lets the first projection matmul
                    # start after ~128KB of input, not the full 1MB
                    xq = [pers.tile([P, 2, SJ], QP_DT, name=f"xq{j}_{i}")
                          for i in range(ND // 2)]
                    xk = [pers.tile([P, 2, SJ], QP_DT, name=f"xk{j}_{i}")
                          for i in range(ND // 2)]
                    for c in range(0, ND, 2):
                        nc.sync.dma_start(
                            xq[c // 2][:],
                            qT_d.ap().rearrange(
                                "(o p) s -> p o s", p=P)[:, c:c + 2, js])
                        nc.sync.dma_start(
                            xk[c // 2][:],
                            kT_d.ap().rearrange(
                                "(o p) s -> p o s", p=P)[:, c:c + 2, js])

                    def xq_ap(c, xq=xq):
                        return xq[c // 2][:, c % 2, :]

                    def xk_ap(c, xk=xk):
                        return xk[c // 2][:, c % 2, :]
                else:
                    xqt = pers.tile([P, ND, SJ], QP_DT, name=f"xq{j}")
                    nc.sync.dma_start(
                        xqt[:],
                        qT_d.ap().rearrange("(o p) s -> p o s", p=P)[:, :, js])
                    xkt = pers.tile([P, ND, SJ], QP_DT, name=f"xk{j}")
                    nc.sync.dma_start(
                        xkt[:],
                        kT_d.ap().rearrange("(o p) s -> p o s", p=P)[:, :, js])

                    def xq_ap(c, xqt=xqt):
                        return xqt[:, c, :]

                    def xk_ap(c, xkt=xkt):
                        return xkt[:, c, :]
                xv = pers.tile([P, ND, SJ], V_DT, name=f"xv{j}")
                nc.sync.dma_start(
                    xv[:], vT_d.ap().rearrange("(o p) s -> p o s", p=P)[:, :, js])
                return xq_ap, xk_ap, xv

            def emit_projQK(j, xq_ap, xk_ap):
                js = slice(j * SJ, (j + 1) * SJ)
                for g in range(2):
                    pq = pj_pool.tile([P, SJ], f32, tag="pj", name=f"pq{j}{g}")
                    for c in range(ND):
                        nc.tensor.matmul(
                            pq[:], wq_sb[c // 2][:, c % 2, bass.ts(g, P)],
                            xq_ap(c),
                            start=(c == 0), stop=(c == ND - 1))
                    nc.vector.tensor_copy(QT[g][:, js], pq[:])
                for g in range(2):
                    pk = pj_pool.tile([P, SJ], f32, tag="pj", name=f"pk{j}{g}")
                    for c in range(ND):
                        nc.tensor.matmul(
                            pk[:], wk_sb[c // 2][:, c % 2, bass.ts(g, P)],
                            xk_ap(c),
                            start=(c == 0), stop=(c == ND - 1))
                    nc.vector.tensor_copy(KTH[g][0][0:E, js], pk[0:E, :])
                    nc.vector.tensor_copy(
                        KTH[g][1][E:2 * E, js], pk[E:2 * E, :])

            def emit_projV(j, xv):
                for u in range(SJ // P):
                    t = 4 * j + u
                    pv = pj_pool.tile([P, 2, 2, E], f32, tag="pj",
                                      name=f"pv{j}{u}")
                    for c in range(ND):
                        nc.tensor.matmul(
                            pv[:], xv[:, c, bass.ts(u, P)], wv_sb[:, c, :],
                            start=(c == 0), stop=(c == ND - 1))
                    for g in range(2):
                        if j <= 1:
                            # early tiles: DVE queue depth gates the epi
                            # den-copies (and with them the atp release);
                            # ACT is lightly loaded here, so move the V
                            # copies there
                            nc.scalar.activation(
                                V[g][:, t, :, 0:E], pv[:, g, :, :],
                                mybir.ActivationFunctionType.Copy, scale=1.0)
                        else:
                            nc.vector.tensor_copy(
                                V[g][:, t, :, 0:E], pv[:, g, :, :])

            def emit_wo(j):
                for u in range(SJ // P):
                    si = 4 * j + u
                    ot = ot_pool.tile([P, D], fp16, tag="ot", name=f"ot{si}")
                    for no in range(2):
                        po = pj_pool.tile([P, SJ], f32, tag="pj",
                                          name=f"po{si}{no}")
                        for gg in range(2):
                            nc.tensor.matmul(
                                po[:], attnG[gg][:, bass.ts(si, P)],
                                wot_sb[:, gg, bass.ts(no, SJ)],
                                start=(gg == 0), stop=(gg == 1))
                        nc.vector.tensor_copy(ot[:, bass.ts(no, SJ)], po[:])
                    nc.sync.dma_start(out_d.ap()[bass.ts(si, P), :], ot[:])

            def emit_woA(j):
                # g=0 half of the last tile's output projection, emitted
                # after attn(1,j) so the scheduler can slot it into the exp
                # bubbles while epi(1,j) is still pending. Written to out2
                # (host adds the two partials), so no PSUM has to survive
                # until the g=1 half.
                for u in range(SJ // P):
                    si = 4 * j + u
                    ota = ot_pool.tile([P, D], fp16, tag="ot", name=f"ota{si}")
                    for no in range(2):
                        po = pj_pool.tile([P, SJ], f32, tag="pj",
                                          name=f"poa{si}{no}")
                        nc.tensor.matmul(
                            po[:], attnG[0][:, bass.ts(si, P)],
                            wot_sb[:, 0, bass.ts(no, SJ)],
                            start=True, stop=True)
                        nc.vector.tensor_copy(ota[:, bass.ts(no, SJ)], po[:])
                    nc.sync.dma_start(out2_d.ap()[bass.ts(u, P), :], ota[:])

            def emit_woB(j):
                # g=1 half, gated only by the fast epilogue; casts alternate
                # DVE/ACT since both are idle in the tail. po banks come
                # from the at pool (its 4 banks are free once the epilogues
                # release the atp pairs) so the 8 matmuls run back-to-back
                # instead of beating against 2 pj slots; output DMAs go per
                # half so the last one isn't gated on both casts.
                for u in range(SJ // P):
                    si = 4 * j + u
                    ot = ot_pool.tile([P, D], fp16, tag="ot", name=f"otb{si}")
                    for no in range(2):
                        po = at_pool.tile([P, SJ], f32, tag="at",
                                          name=f"pob{si}{no}")
                        nc.tensor.matmul(
                            po[:], attnG[1][:, bass.ts(si, P)],
                            wot_sb[:, 1, bass.ts(no, SJ)],
                            start=True, stop=True)
                        if no == 0:
                            nc.vector.tensor_copy(ot[:, bass.ts(no, SJ)], po[:])
                        else:
                            nc.scalar.activation(
                                ot[:, bass.ts(no, SJ)], po[:],
                                mybir.ActivationFunctionType.Copy, scale=1.0)
                        nc.sync.dma_start(
                            out_d.ap()[bass.ts(si, P), bass.ts(no, SJ)],
                            ot[:, bass.ts(no, SJ)])

            def emit_attn(g, j):
                nblk = 4 * j + 4
                atp = [
                    at_pool.tile([P, SJ], f32, tag="at", name=f"at{g}{j}{h2}")
                    for h2 in range(2)
                ]
                for cb in range(nblk):
                    col0 = max(0, cb - 4 * j) * P
                    # both heads' score matmuls back to back; per-head exp
                    # keeps the PE/ACT pipeline beat fine-grained (a merged
                    # 2-bank ACTIVATE was tried: it saves ~20us of ACT time
                    # but stalls the PE ~1.2us per block and re-triggers the
                    # HAM clock throttle -- net loss)
                    scps = []
                    for h2 in range(2):
                        scp = sc_pool.tile(
                            [P, SJ], f32, tag="sc", name=f"sc{g}{j}{cb}{h2}")
                        nc.tensor.matmul(
                            scp[:, col0:],
                            KTH[g][h2][:, bass.ts(cb, P)],
                            QT[g][:, j * SJ + col0:(j + 1) * SJ],
                            start=True, stop=True)
                        scps.append(scp)
                    for h2 in range(2):
                        ex = ex_pool.tile(
                            [P, SJ], V_DT, tag="ex", name=f"ex{g}{j}{cb}{h2}")
                        nc.scalar.activation(
                            ex[:, col0:], scps[h2][:, col0:], EXP,
                            scale=1.0 / (32.0 * WSCALE * WSCALE))
                        if cb >= 4 * j:
                            nc.vector.tensor_tensor(
                                ex[:, col0:col0 + P], ex[:, col0:col0 + P],
                                tri_sb[:], MULT)
                        nc.tensor.matmul(
                            atp[h2][0:E + 1, col0:],
                            V[g][:, cb, h2, :],
                            ex[:, col0:],
                            start=(cb == 0), stop=(cb == nblk - 1))
                return atp

            def emit_epi(g, j, atp):
                # normalize by the softmax denominators (PSUM row 64):
                # approx-fast reciprocal (fp32, ~18 bits), gpsimd broadcasts
                # the row over the value partitions, one multiply per head.
                # No PE ops in this chain.
                # (Tried and rejected: staging atp values to SBUF to release
                # the PSUM pair early: +8us of DVE queue depth; PE-matmul
                # den broadcast mid-loop: +36us -- the K=1 matmuls block the
                # in-order PE queue while waiting on den copies; DVE divide:
                # fails the TRN2 ISA check; PSUM DMA endpoints: forbidden.)
                js = slice(j * SJ, (j + 1) * SJ)
                den = sm_pool.tile([E + 1, 2, SJ], f32, tag="den",
                                   name=f"den{g}{j}")
                for h2 in range(2):
                    # custom-DVE reciprocal reads garbage from PSUM on HW
                    # (sim allows it) -- stage through SBUF
                    nc.vector.tensor_copy(
                        den[E:E + 1, h2, :], atp[h2][E:E + 1, :])
                # DMA hops the rows to partition 0 (DVE and the custom recip
                # cannot cross partition bases; partition_broadcast reads its
                # source from partition 0 only). j=0's DMAs ride the scalar
                # queue: the weights are loaded by then and it keeps the
                # gpsimd queue short exactly where epi(1,0)'s release of the
                # atp pair gates attn(1,1).
                dq = nc.scalar if j == 0 else nc.gpsimd
                rec0 = sm_pool.tile([1, 2, SJ], f32, tag="rec0",
                                    name=f"rec0{g}{j}")
                dq.dma_start(rec0[:], den[E:E + 1, :, :])
                recr = sm_pool.tile([1, 2, SJ], f32, tag="recr",
                                    name=f"recr{g}{j}")
                nc.vector.reciprocal_approx_fast(out=recr[:], in_=rec0[:])
                recb = [sm_pool.tile([E, SJ], f32, tag=f"recb{h2}",
                                     name=f"recb{g}{j}{h2}")
                        for h2 in range(2)]
                for h2 in range(2):
                    nc.gpsimd.partition_broadcast(
                        recb[h2][:], recr[0:1, h2, :])
                nc.vector.tensor_tensor(
                    attnG[g][0:E, js], atp[0][0:E, :], recb[0][:], MULT)
                ah = sm_pool.tile([E, SJ], V_DT, tag="ah", name=f"ah{g}{j}")
                nc.vector.tensor_tensor(
                    ah[:], atp[1][0:E, :], recb[1][:], MULT)
                dq.dma_start(attnG[g][E:2 * E, js], ah[:])

            def emit_epi_fast(g, j, atp, psum_pool, psum_tag):
                # Last-tile epilogue, latency-optimized: den row -> SBUF
                # (partition 64), K=1 PE matmul broadcasts it over the 64
                # value partitions into PSUM (the PE is idle at the tail, so
                # this is free AND keeps it out of the idle-triggered
                # half-clock p-state), ACT Copy bridges back to SBUF (DVE
                # reads only one PSUM operand; Copy co-resides with Exp in
                # the act table), DVE reciprocal on the broadcast (base 0),
                # one multiply per head. Skips the partition-0 DMA hop +
                # gpsimd broadcast chain (~7us serial).
                js = slice(j * SJ, (j + 1) * SJ)
                den = sm_pool.tile([E + 1, 2, SJ], bf16, tag="denf",
                                   name=f"denf{g}{j}")
                # h1 on DVE, h0 on ACT: the two den-row copies run in
                # parallel instead of serializing on DVE
                nc.vector.tensor_copy(den[E:E + 1, 1, :], atp[1][E:E + 1, :])
                nc.scalar.activation(
                    den[E:E + 1, 0, :], atp[0][E:E + 1, :],
                    mybir.ActivationFunctionType.Copy, scale=1.0)
                recb = [None, None]
                for h2 in (1, 0):
                    db = psum_pool.tile([E, SJ], f32, tag=psum_tag,
                                        name=f"denb{g}{j}{h2}")
                    nc.tensor.matmul(
                        db[:], ones_sb[E:E + 1, :], den[E:E + 1, h2, :],
                        start=True, stop=True)
                    dbs = sm_pool.tile([E, SJ], f32, tag=f"recb{h2}",
                                       name=f"denbs{g}{j}{h2}")
                    nc.scalar.activation(
                        dbs[:], db[:],
                        mybir.ActivationFunctionType.Copy, scale=1.0)
                    rb = sm_pool.tile([E, SJ], f32, tag=f"recc{h2}",
                                      name=f"recbf{g}{j}{h2}")
                    nc.vector.reciprocal_approx_fast(out=rb[:], in_=dbs[:])
                    recb[h2] = rb
                ah = sm_pool.tile([E, SJ], V_DT, tag="ah", name=f"ahf{g}{j}")
                nc.vector.tensor_tensor(
                    ah[:], atp[1][0:E, :], recb[1][:], MULT)
                # halved so the first woB matmuls start sooner
                for half in range(2):
                    lo = j * SJ + half * (SJ // 2)
                    nc.gpsimd.dma_start(
                        attnG[g][E:2 * E, lo:lo + SJ // 2],
                        ah[:, half * (SJ // 2):(half + 1) * (SJ // 2)])
                nc.vector.tensor_tensor(
                    attnG[g][0:E, js], atp[0][0:E, :], recb[0][:], MULT)

            loads = {j: emit_loads(j) for j in range(NJ)}
            for j in range(NJ):
                xq_ap, xk_ap, xv = loads.pop(j)
                emit_projQK(j, xq_ap, xk_ap)
                emit_projV(j, xv)
                atp = emit_attn(0, j)
                if j < NJ - 1:
                    emit_epi(0, j, atp)
                else:
                    # fast epi for (0, last) too: attnG[0] completes a few us
                    # into attn(1,3), so woA can hoist into its exp bubbles
                    # instead of waiting ~15us of gpsimd queue. pj banks: sc
                    # is hot with attn(1,3) scores, pj only has wo(2) po's.
                    emit_epi_fast(0, j, atp, pj_pool, "pj")
                atp = emit_attn(1, j)
                if j < NJ - 1:
                    emit_epi(1, j, atp)
                    if j >= 1:
                        emit_wo(j - 1)
                else:
                    emit_wo(j - 1)
                    emit_woA(j)
                    emit_epi_fast(1, j, atp, sc_pool, "sc")
                    emit_woB(j)

    nc.compile()
    return nc


def _get_nc():
    if not _NC_CACHE:
        _NC_CACHE.append(_build())
    return _NC_CACHE[0]


def _in_maps(q, k, v, W_q, W_k, W_v, W_o):
    qp_np = _NP_OF[QP_DT]
    v_np = _NP_OF[V_DT]
    tri = (np.arange(P)[:, None] <= np.arange(P)[None, :]).astype(v_np)
    xT = {}
    for b in range(B):
        xT[b] = (
            np.ascontiguousarray(q[b].T).astype(qp_np),
            np.ascontiguousarray(k[b].T).astype(qp_np),
            np.ascontiguousarray(v[b].T).astype(v_np),
        )
    maps = []
    for core in range(NCORES):
        b, quad = divmod(core, 4)
        hs = slice(4 * quad, 4 * quad + 4)
        qT_b, kT_b, vT_b = xT[b]
        maps.append({
            "qT": qT_b,
            "kT": kT_b,
            "vT": vT_b,
            # [4, D, E] -> [D, 4, E] -> [D, 256], col l*64+e = W[4q+l, d, e]
            # prescaled by WSCALE to stay clear of e4m3 subnormals
            "wq": np.ascontiguousarray(
                W_q[hs].transpose(1, 0, 2).reshape(D, 4 * E)
                * WSCALE).astype(qp_np),
            "wk": np.ascontiguousarray(
                W_k[hs].transpose(1, 0, 2).reshape(D, 4 * E)
                * WSCALE).astype(qp_np),
            "wv": np.ascontiguousarray(
                W_v[hs].transpose(1, 0, 2).reshape(D, 4 * E)).astype(v_np),
            # W_o[out, in] -> W_o.T rows for this quad's 256 input dims
            "wot": np.ascontiguousarray(
                W_o[:, 4 * quad * E:4 * quad * E + 4 * E].T).astype(v_np),
            "tri": tri,
        })
    return maps


def kernel(q, k, v, W_q, W_k, W_v, W_o, _trace=False, _trace_kwargs=None):
    q = np.asarray(q, dtype=np.float32)
    k = np.asarray(k, dtype=np.float32)
    v = np.asarray(v, dtype=np.float32)
    W_q = np.asarray(W_q, dtype=np.float32)
    W_k = np.asarray(W_k, dtype=np.float32)
    W_v = np.asarray(W_v, dtype=np.float32)
    W_o = np.asarray(W_o, dtype=np.float32)

    nc = _get_nc()
    maps = _in_maps(q, k, v, W_q, W_k, W_v, W_o)
    kwargs = dict(_trace_kwargs or {})
    res = run_bass_kernel_spmd(
        nc, maps, core_ids=list(range(NCORES)), trace=_trace, **kwargs)
    out = np.zeros((B, S, D), dtype=np.float32)
    for core in range(NCORES):
        b = core // 4
        out[b] += res.results[core]["out"].astype(np.float32)
        out[b, (NJ - 1) * SJ:] += res.results[core]["out2"].astype(np.float32)
    if _trace:
        kernel.last_results = res
    return out

